# revision 1
# baseline (speedup 1.0000x reference)
"""Trainium2 Bass kernel for ConcatMLPAggregator (topk_masking).

Pipeline per core (4096 chains, data-parallel over 8 cores):
  mask -> scores = mask * (64 - pos) -> DVE Max8/MaxIndex -> first-8 masked
  positions per chain -> v-row offsets (unpicked -> zero row appended to v)
  -> indirect-DMA gather (512B rows) -> PE transpose chunks -> x^T
  -> mm1: W1 stationary, chains streamed (+ K=1 log1p(count) term)
  -> GELU+b1 on ScalarE -> mm2: h^T chunks stationary, W2 streamed
  (+ K=1 b2 term) -> out.

Chain mapping: c = p*32 + j  (p = SBUF partition, j = free slot) so that
batch_idx/count/output DMAs use contiguous per-partition runs.
"""

import sys

for _p in ("/opt/trn_rl_repo",):
    if _p not in sys.path:
        sys.path.insert(0, _p)

import numpy as np

N_CORES = 8
N_CHAINS = 32768
CPC = N_CHAINS // N_CORES  # 4096 chains per core
L = 64
DV = 128
KSET = 8
HID = 128
N_ITEMS = 8192
N_ROWS = N_ITEMS * L  # 524288 rows of v
ZROW = N_ROWS  # index of the appended zero row
IN_DIM = KSET * DV + 1  # 1025

N_TILES = CPC // 128  # 32 tiles of 128 chains
TILES_PER_SLAB = 4
N_SLABS = N_TILES // TILES_PER_SLAB  # 8 slabs of 512 chains
SLAB = 128 * TILES_PER_SLAB  # 512


def _emit(nc, tc, ctx, aps, dbg=None):
    import concourse.bass as bass
    from concourse import mybir
    from concourse.masks import make_identity

    fp32 = mybir.dt.float32
    i32 = mybir.dt.int32
    u32 = mybir.dt.uint32
    u8 = mybir.dt.uint8
    Alu = mybir.AluOpType
    Act = mybir.ActivationFunctionType

    v = aps["v"]          # [N_ROWS+1, DV] fp32 (zero row appended)
    maskb = aps["maskb"]  # [CPC, L] uint8
    bidx = aps["bidx"]    # [CPC] int32
    cnt = aps["cnt"]      # [CPC] int32
    W1 = aps["W1"]        # [IN_DIM, HID] fp32
    b1 = aps["b1"]        # [HID]
    W2 = aps["W2"]        # [HID, DV]
    b2 = aps["b2"]        # [DV]
    out = aps["out"]      # [CPC, DV] fp32

    const = ctx.enter_context(tc.tile_pool(name="const", bufs=1))
    small = ctx.enter_context(tc.tile_pool(name="small", bufs=2))
    xg_pool = ctx.enter_context(tc.tile_pool(name="xg", bufs=2))
    xt_pool = ctx.enter_context(tc.tile_pool(name="xt", bufs=2))
    ht_pool = ctx.enter_context(tc.tile_pool(name="ht", bufs=2))
    ot_pool = ctx.enter_context(tc.tile_pool(name="ot", bufs=2))
    ps_xt = ctx.enter_context(tc.tile_pool(name="ps_xt", bufs=2, space="PSUM"))
    ps_h = ctx.enter_context(tc.tile_pool(name="ps_h", bufs=2, space="PSUM"))
    ps_o = ctx.enter_context(tc.tile_pool(name="ps_o", bufs=2, space="PSUM"))

    # ---- constants ----
    ident = const.tile([128, 128], fp32)
    make_identity(nc, ident[:])

    w1sb = const.tile([128, 8 * HID], fp32)  # chunk k at free [k*128, +128)
    nc.sync.dma_start(
        out=w1sb[:].rearrange("p (k h) -> p k h", k=8),
        in_=W1[: 8 * 128, :].rearrange("(k p) h -> p k h", p=128),
    )
    w1last = const.tile([1, HID], fp32)
    nc.sync.dma_start(out=w1last[:], in_=W1[8 * 128 :, :])
    w2sb = const.tile([128, DV], fp32)
    nc.sync.dma_start(out=w2sb[:], in_=W2[:, :])
    b1col = const.tile([128, 1], fp32)
    nc.sync.dma_start(out=b1col[:], in_=b1.rearrange("(p o) -> p o", o=1))
    b2row = const.tile([1, DV], fp32)
    nc.sync.dma_start(out=b2row[:], in_=b2.rearrange("(o h) -> o h", o=1))
    ones1 = const.tile([1, 128], fp32)
    nc.vector.memset(ones1[:], 1.0)

    # desc[p, j*L + l] = 64 - l  (scores weight, same on every partition)
    desc_i = const.tile([128, TILES_PER_SLAB * L], i32)
    nc.gpsimd.iota(
        desc_i[:],
        pattern=[[0, TILES_PER_SLAB], [-1, L]],
        base=L,
        channel_multiplier=0,
    )
    desc_f = const.tile([128, TILES_PER_SLAB * L], fp32)
    nc.vector.tensor_copy(desc_f[:], desc_i[:])

    # ---- per-core small tensors ----
    b_sb = const.tile([128, N_TILES], i32)  # b_sb[p, j] = batch_idx[p*32+j]
    nc.sync.dma_start(out=b_sb[:], in_=bidx.rearrange("(p j) -> p j", j=N_TILES))
    c_sb = const.tile([128, N_TILES], i32)
    nc.sync.dma_start(out=c_sb[:], in_=cnt.rearrange("(p j) -> p j", j=N_TILES))

    # log1p(count), then scatter to a single row [1, CPC] ordered by c'=(j*128+p)
    c_f = const.tile([128, N_TILES], fp32)
    nc.vector.tensor_copy(c_f[:], c_sb[:])
    lc_pm = const.tile([128, N_TILES], fp32)
    nc.scalar.activation(lc_pm[:], c_f[:], Act.Ln, bias=1.0)
    # Stage through DRAM to reorder [p, j] -> flat row ordered by c' = j*128+p
    lcstage = aps["lcstage"]
    nc.sync.dma_start(
        out=lcstage.rearrange("(p j) -> p j", j=N_TILES), in_=lc_pm[:]
    )
    logct = const.tile([1, CPC], fp32)
    nc.sync.dma_start(
        out=logct[:], in_=lcstage.rearrange("(p j) -> j p", j=N_TILES)
    )

    mask_r = maskb.rearrange("(p j) l -> p j l", j=N_TILES)
    out_r = out.rearrange("(p j) d -> p j d", j=N_TILES)

    for s in range(N_SLABS):
        # ---- mask -> scores -> top8 indices ----
        msk = small.tile([128, TILES_PER_SLAB * L], u8, tag="msk")
        nc.sync.dma_start(
            out=msk[:].rearrange("p (j l) -> p j l", l=L),
            in_=mask_r[:, s * TILES_PER_SLAB : (s + 1) * TILES_PER_SLAB, :],
        )
        vals = small.tile([128, TILES_PER_SLAB * KSET], fp32, tag="vals")
        idx = small.tile([128, TILES_PER_SLAB * KSET], u32, tag="idx")
        scor = small.tile([128, TILES_PER_SLAB * L], fp32, tag="scor")
        nc.vector.scalar_tensor_tensor(
            out=scor[:], in0=msk[:], scalar=1.0, in1=desc_f[:],
            op0=Alu.mult, op1=Alu.mult,
        )
        for jl in range(TILES_PER_SLAB):
            sc = scor[:, jl * L : (jl + 1) * L]
            nc.vector.max(vals[:, jl * KSET : (jl + 1) * KSET], sc)
            nc.vector.max_index(
                idx[:, jl * KSET : (jl + 1) * KSET],
                vals[:, jl * KSET : (jl + 1) * KSET],
                sc,
            )

        # ---- offsets: picked ? b*64 + idx : ZROW ----
        offs = small.tile([128, TILES_PER_SLAB * KSET], i32, tag="offs")
        bb = b_sb[:, s * TILES_PER_SLAB : (s + 1) * TILES_PER_SLAB]
        nc.vector.tensor_scalar(
            out=offs[:].rearrange("p (j k) -> p j k", k=KSET),
            in0=bb.to_broadcast([128, TILES_PER_SLAB, KSET]),
            scalar1=L,
            scalar2=None,
            op0=Alu.mult,
        )
        nc.vector.scalar_tensor_tensor(
            out=offs[:], in0=offs[:], scalar=0, in1=idx[:].bitcast(i32),
            op0=Alu.add, op1=Alu.add,
        )
        picked = small.tile([128, TILES_PER_SLAB * KSET], u8, tag="picked")
        nc.vector.tensor_scalar(
            out=picked[:], in0=vals[:], scalar1=0.0, scalar2=None, op0=Alu.is_gt
        )
        offz = small.tile([128, TILES_PER_SLAB * KSET], i32, tag="offz")
        nc.vector.memset(offz[:], ZROW)
        nc.vector.copy_predicated(out=offz[:], mask=picked[:], data=offs[:])

        # ---- gather: xg[p, jl*1024 + k*128 + f] = v[offz[p, jl*8+k], f] ----
        xg = xg_pool.tile([128, TILES_PER_SLAB * KSET * DV], fp32)
        for l in range(TILES_PER_SLAB * KSET):
            nc.gpsimd.indirect_dma_start(
                out=xg[:, l * DV : (l + 1) * DV],
                out_offset=None,
                in_=v[:, :],
                in_offset=bass.IndirectOffsetOnAxis(ap=offz[:, l : l + 1], axis=0),
            )

        # ---- transpose x chunks: PE matmul-with-identity, copies split DVE/ACT
        xt = xt_pool.tile([128, KSET * SLAB], fp32)  # chunk k at [k*512,+512)
        for k in range(KSET):
            pxt = ps_xt.tile([128, SLAB], fp32)
            for jl in range(TILES_PER_SLAB):
                nc.tensor.transpose(
                    out=pxt[:, jl * 128 : (jl + 1) * 128],
                    in_=xg[:, jl * KSET * DV + k * DV : jl * KSET * DV + (k + 1) * DV],
                    identity=ident[:],
                )
            dst = xt[:, k * SLAB : (k + 1) * SLAB]
            if k % 2 == 0:
                nc.vector.tensor_copy(dst, pxt[:])
            else:
                nc.scalar.copy(dst, pxt[:])

        # ---- mm1: h^T[h, c'] = sum_f W1[f,h] * x[c',f] (+ log1p count term) --
        ph = ps_h.tile([128, SLAB], fp32)
        for k in range(KSET):
            nc.tensor.matmul(
                ph[:],
                lhsT=w1sb[:, k * 128 : (k + 1) * 128],
                rhs=xt[:, k * SLAB : (k + 1) * SLAB],
                start=(k == 0),
                stop=False,
            )
        nc.tensor.matmul(
            ph[:],
            lhsT=w1last[:, :],
            rhs=logct[:, s * SLAB : (s + 1) * SLAB],
            start=False,
            stop=True,
        )

        # ---- gelu(+b1) ----
        ht = ht_pool.tile([128, SLAB], fp32)
        nc.scalar.activation(ht[:], ph[:], Act.Gelu, bias=b1col[:, :1])

        # ---- mm2: out[c, d] = sum_h h[c,h] W2[h,d] + b2 ----
        po = ps_o.tile([128, SLAB], fp32)
        for ts in range(TILES_PER_SLAB):
            dst = po[:, ts * 128 : (ts + 1) * 128]
            nc.tensor.matmul(
                dst,
                lhsT=ht[:, ts * 128 : (ts + 1) * 128],
                rhs=w2sb[:],
                start=True,
                stop=False,
            )
            nc.tensor.matmul(
                dst, lhsT=ones1[:, :], rhs=b2row[:, :], start=False, stop=True
            )
        osb = ot_pool.tile([128, SLAB], fp32)
        nc.vector.tensor_copy(osb[:], po[:])
        nc.sync.dma_start(
            out=out_r[:, s * TILES_PER_SLAB : (s + 1) * TILES_PER_SLAB, :],
            in_=osb[:].rearrange("p (j d) -> p j d", d=DV),
        )

        if dbg is not None:
            w = TILES_PER_SLAB * KSET
            nc.sync.dma_start(out=dbg["vals"][:, s * w : (s + 1) * w], in_=vals[:])
            nc.sync.dma_start(out=dbg["idx"][:, s * w : (s + 1) * w], in_=idx[:])
            nc.sync.dma_start(out=dbg["offz"][:, s * w : (s + 1) * w], in_=offz[:])
            wg = TILES_PER_SLAB * KSET * DV
            nc.sync.dma_start(out=dbg["xg"][:, s * wg : (s + 1) * wg], in_=xg[:])
            nc.sync.dma_start(
                out=dbg["xt"][:, s * wg : (s + 1) * wg], in_=xt[:]
            )
            nc.sync.dma_start(
                out=dbg["ht"][:, s * SLAB : (s + 1) * SLAB], in_=ht[:]
            )
    if dbg is not None:
        nc.sync.dma_start(out=dbg["logct"][:], in_=logct[:])


def _legalize_waits(nc, max_inline=1):
    """This container's walrus rejects instructions with >1 sync wait.

    Hoist extra waits into standalone EventSemaphore instructions on the
    same engine, placed immediately before the instruction.
    """
    from concourse import mybir

    n = 0
    for func in nc.m.functions:
        for block in func.blocks:
            new_insts = []
            for inst in block.instructions:
                si = inst.sync_info
                if si is not None and len(si.on_wait) > max_inline:
                    waits = list(si.on_wait)
                    extra, keep = waits[:-max_inline], waits[-max_inline:]
                    for w in extra:
                        n += 1
                        new_insts.append(
                            mybir.InstEventSemaphore(
                                name=f"hoistw_{n}_{inst.name}",
                                engine=inst.engine,
                                ins=[],
                                outs=[],
                                sync_info=mybir.SyncInfo(
                                    on_wait=[w], on_update=[]
                                ),
                            )
                        )
                    si.on_wait = keep
                new_insts.append(inst)
            block.instructions[:] = new_insts


def build_program(debug_dump=False, legalize=True):
    from contextlib import ExitStack

    import concourse.bass as bass
    import concourse.tile as tile
    from concourse import mybir

    fp32 = mybir.dt.float32
    i32 = mybir.dt.int32
    u32 = mybir.dt.uint32
    u8 = mybir.dt.uint8

    nc = bass.Bass("TRN2", target_bir_lowering=False, debug=False)
    aps = {
        "v": nc.dram_tensor("v", [N_ROWS + 1, DV], fp32, kind="ExternalInput").ap(),
        "maskb": nc.dram_tensor("maskb", [CPC, L], u8, kind="ExternalInput").ap(),
        "bidx": nc.dram_tensor("bidx", [CPC], i32, kind="ExternalInput").ap(),
        "cnt": nc.dram_tensor("cnt", [CPC], i32, kind="ExternalInput").ap(),
        "W1": nc.dram_tensor("W1", [IN_DIM, HID], fp32, kind="ExternalInput").ap(),
        "b1": nc.dram_tensor("b1", [HID], fp32, kind="ExternalInput").ap(),
        "W2": nc.dram_tensor("W2", [HID, DV], fp32, kind="ExternalInput").ap(),
        "b2": nc.dram_tensor("b2", [DV], fp32, kind="ExternalInput").ap(),
        "out": nc.dram_tensor("out", [CPC, DV], fp32, kind="ExternalOutput").ap(),
        "lcstage": nc.dram_tensor("lcstage", [CPC], fp32).ap(),
    }
    dbg = None
    if debug_dump:
        W = N_TILES * KSET
        dbg = {
            "vals": nc.dram_tensor("dbg_vals", [128, W], fp32, kind="ExternalOutput").ap(),
            "idx": nc.dram_tensor("dbg_idx", [128, W], u32, kind="ExternalOutput").ap(),
            "offz": nc.dram_tensor("dbg_offz", [128, W], i32, kind="ExternalOutput").ap(),
            "xg": nc.dram_tensor("dbg_xg", [128, W * DV], fp32, kind="ExternalOutput").ap(),
            "xt": nc.dram_tensor("dbg_xt", [128, W * DV], fp32, kind="ExternalOutput").ap(),
            "ht": nc.dram_tensor("dbg_ht", [128, CPC], fp32, kind="ExternalOutput").ap(),
            "logct": nc.dram_tensor("dbg_logct", [1, CPC], fp32, kind="ExternalOutput").ap(),
        }
    with tile.TileContext(nc) as tc:
        with ExitStack() as ctx:
            _emit(nc, tc, ctx, aps, dbg=dbg)
    if legalize:
        _legalize_waits(nc)
    return nc


def make_in_maps(v, batch_idx, mask, count, W1, b1, W2, b2):
    vpad = np.concatenate(
        [np.ascontiguousarray(v, dtype=np.float32).reshape(N_ROWS, DV),
         np.zeros((1, DV), dtype=np.float32)],
        axis=0,
    )
    mask_u8 = np.ascontiguousarray(mask).view(np.uint8).reshape(N_CHAINS, L)
    in_maps = []
    for c in range(N_CORES):
        sl = slice(c * CPC, (c + 1) * CPC)
        in_maps.append(
            {
                "v": vpad,
                "maskb": np.ascontiguousarray(mask_u8[sl]),
                "bidx": np.ascontiguousarray(batch_idx[sl], dtype=np.int32),
                "cnt": np.ascontiguousarray(count[sl], dtype=np.int32),
                "W1": np.ascontiguousarray(W1, dtype=np.float32),
                "b1": np.ascontiguousarray(b1, dtype=np.float32),
                "W2": np.ascontiguousarray(W2, dtype=np.float32),
                "b2": np.ascontiguousarray(b2, dtype=np.float32),
            }
        )
    return in_maps


def run(in_maps, trace=False):
    from concourse.bass_utils import run_bass_kernel_spmd

    nc = build_program()
    res = run_bass_kernel_spmd(
        nc, in_maps, list(range(N_CORES)), trace=trace
    )
    outs = [r["out"] for r in res.results]
    return np.concatenate(outs, axis=0), res


def kernel(v, batch_idx, mask, count, W1, b1, W2, b2):
    in_maps = make_in_maps(v, batch_idx, mask, count, W1, b1, W2, b2)
    out, _ = run(in_maps, trace=False)
    return out

